# revision 1
# baseline (speedup 1.0000x reference)
"""Trainium2 Bass kernel for nn_NisuyNN_90434831384984.

Math: the reference's stack+reshape makes MLP row (s,t,b) depend only on s
(b in {0,1}) or only on t (b in {2,3}), and rows for b=2,3 equal those for
b=0,1 — so the 4096-row x 6-layer MLP collapses to 64 unique rows producing
64 unique 32x32 policy matrices.  The 50-step power iteration (eigengap
~0.012) is replaced by 8 unnormalized steps (converged below fp32 eps; the
final deltas use only intra-vector ratios, so the scale cancels).

Distribution: Megatron-style column-split of every layer across 8 cores,
with per-layer AllGathers of the locally transposed activation slice.
Each layer's output columns are split into G=2 halves so each half's
AllGather overlaps the other half's matmuls (and the next layer starts
on gathered half-0 K-chunks while half-1 is still in flight).  Weights
are sliced on the host and streamed as bf16; PSUM accumulation is fp32;
the eigensolve + deltas tail stays fp32.
"""

import numpy as np

DIM = 128
N = 32
B = 4
H = 4096
NC = 8          # cores
SL = H // NC    # 512 hidden slice per core
OF = N * N      # 1024 output features
OSL = OF // NC  # 128 output slice per core
R = 64          # unique MLP rows
KC = 128        # contraction chunk
G = 1           # column groups per layer (G>1 pipelines AGs but doubles
                # the per-collective CC-core serial floor; G=1 measured best)
PI_ITERS = 7    # extra matvec iterations after the init row-sum step
SLOPE = 0.01

_COMPILED = None
LAST_RESULTS = None


def _build_body(nc, tc, tile, mybir, aps):
    f32 = mybir.dt.float32
    bf16 = mybir.dt.bfloat16
    AF = mybir.ActivationFunctionType
    ALU = mybir.AluOpType
    AX = mybir.AxisListType
    rg = [list(range(NC))]

    from contextlib import ExitStack
    es = ExitStack()
    cpool = es.enter_context(tc.tile_pool(name="consts", bufs=1))
    wpool = es.enter_context(tc.tile_pool(name="w", bufs=20))
    bpool = es.enter_context(tc.tile_pool(name="b", bufs=2))
    apool = es.enter_context(tc.tile_pool(name="act", bufs=2))
    atp = es.enter_context(tc.tile_pool(name="atT", bufs=2))
    lpool = es.enter_context(tc.tile_pool(name="lhs", bufs=3))
    pipool = es.enter_context(tc.tile_pool(name="pi", bufs=2))
    tailp = es.enter_context(tc.tile_pool(name="tail", bufs=1))
    ps = es.enter_context(tc.tile_pool(name="ps", bufs=3, space="PSUM"))
    pst = es.enter_context(tc.tile_pool(name="pst", bufs=2, space="PSUM"))
    dram = es.enter_context(tc.tile_pool(name="dram", bufs=3, space="DRAM"))

    # ---- constants ----
    id64 = cpool.tile([64, 64], bf16)
    nc.gpsimd.dma_start(id64[:], aps["ID64"][:])
    dmask = cpool.tile([R, N], f32)
    nc.gpsimd.dma_start(dmask[:], aps["DMASK"][:])
    t01 = cpool.tile([R, N], f32)
    nc.gpsimd.dma_start(t01[:], aps["T01"][:])
    tt23 = cpool.tile([R, N], f32)
    nc.gpsimd.dma_start(tt23[:], aps["TT23"][:])
    mac = cpool.tile([R, 2], f32)
    nc.gpsimd.dma_start(mac[:], aps["MAC"][:])
    ones = cpool.tile([1, R], bf16)
    nc.vector.memset(ones[:], 1.0)

    # Warm up the collective path: the first collective on silicon pays a
    # ~50us one-time init; absorb it behind the initial weight DMAs with a
    # tiny dummy AllGather whose result feeds an (ignored) external output.
    warm_sb = cpool.tile([KC, 8], bf16)
    nc.vector.memset(warm_sb[:], 0.0)
    warm_in = dram.tile([KC, 8], bf16, tag="warm_in")
    nc.gpsimd.dma_start(warm_in[:], warm_sb[:])
    warm_out = dram.tile([NC * KC, 8], bf16, tag="warm_out", addr_space="Shared")
    nc.gpsimd.collective_compute(
        "AllGather", ALU.bypass, replica_groups=rg,
        ins=[warm_in[:].opt()], outs=[warm_out[:].opt()],
    )
    nc.gpsimd.dma_start(aps["warm"][:], warm_out[0:1, :])

    WCH = 4  # K-chunks per weight tile (0.5 MB pieces keep the DMA path
             # available for the latency-critical gather transfers)

    def load_w(w_ap, nk, width):
        """Stream [nk*128, width] weights as ceil(nk/WCH) chunk tiles."""
        wtiles = []
        for wc in range(0, nk, WCH):
            n = min(WCH, nk - wc)
            wt = wpool.tile([KC, n * width], bf16, tag="w")
            nc.sync.dma_start(
                wt[:].rearrange("p (c n) -> p c n", n=width),
                w_ap[wc * KC:(wc + n) * KC, :].rearrange(
                    "(c p) n -> p c n", p=KC),
            )
            wtiles.append(wt)
        return wtiles

    def half_open(parts, wtiles, btile, width, hw, g, bofs=0):
        """Accumulate output columns [g*hw, (g+1)*hw) over all K-chunk parts.
        parts: list of (lhs_tile, ks) with chunk i of lhs_tile having global
        K-chunk index ks[i] into the chunked weight tiles.  K-chunks
        alternate between the two 64-wide PE column groups (tile_position)
        so two matmuls stream concurrently; psum rows [0:64] and [64:128]
        hold the two partial sums.  Returns the [128, hw] psum tile."""
        pt = ps.tile([2 * R, hw], f32, tag="ps")
        first = [True, True]
        cnt = 0
        n_by_half = [0, 0]
        tot = sum(len(ks) for _, ks in parts)
        for lhs, ks in parts:
            for i, k in enumerate(ks):
                h = cnt % 2
                n_by_half[h] += 1
                is_last_of_h1 = (h == 1) and (n_by_half[1] == tot // 2)
                wt = wtiles[k // WCH]
                kc = k % WCH
                nc.tensor.matmul(
                    pt[h * R:(h + 1) * R, :],
                    lhs[:, i * R:(i + 1) * R],
                    wt[:, kc * width + g * hw: kc * width + (g + 1) * hw],
                    start=first[h],
                    stop=is_last_of_h1,
                    tile_position=(0, h * R),
                    skip_group_check=True,
                )
                first[h] = False
                cnt += 1
        nc.tensor.matmul(
            pt[0:R, :], ones[0:1, :], btile[0:1, bofs:bofs + hw],
            start=first[0], stop=True, tile_position=(0, 0),
            skip_group_check=True,
        )
        return pt

    def sum_halves(pt, hw):
        h0 = apool.tile([R, hw], f32, tag="h0")
        nc.scalar.activation(h0[:], pt[0:R, :], AF.Copy)
        z = apool.tile([R, hw], f32, tag="zsum")
        nc.vector.tensor_tensor(z[:], h0[:], pt[R:2 * R, :], op=ALU.add)
        return z

    def lrelu_half(pt, hw):
        z = sum_halves(pt, hw)
        sc = apool.tile([R, hw], f32, tag="lrelu_sc")
        nc.vector.tensor_scalar_mul(sc[:], z[:], SLOPE)
        act = apool.tile([R, hw], bf16, tag="act")
        nc.vector.tensor_tensor(act[:], z[:], sc[:], op=ALU.max)
        return act

    agin_insts = []

    def gather_half(act, width, hw, g):
        """Transpose the [R, hw] half, AllGather; return (lt, ks) for the
        next layer.  Global next-layer K-chunk for (rank, j): f = rank*width
        + g*hw + j*128."""
        nj = hw // KC
        att = atp.tile([KC, nj * R], bf16, tag="atT")
        for j in range(nj):
            tp = pst.tile([KC, R], bf16, tag="pst")
            nc.tensor.transpose(tp[:], act[:, j * KC:(j + 1) * KC], id64[:])
            nc.vector.tensor_copy(att[:, j * R:(j + 1) * R], tp[:])
        ag_in = dram.tile([KC, nj * R], bf16, tag="agin")
        agin_insts.append(nc.scalar.dma_start(ag_in[:], att[:]))
        ag_out = dram.tile([NC * KC, nj * R], bf16, tag="agout",
                           addr_space="Shared")
        nc.gpsimd.collective_compute(
            "AllGather", ALU.bypass, replica_groups=rg,
            ins=[ag_in[:].opt()], outs=[ag_out[:].opt()],
        )
        lt = lpool.tile([KC, NC * nj * R], bf16, tag="lhs")
        nc.scalar.dma_start(
            lt[:].rearrange("p (r j q) -> p r j q", r=NC, q=R),
            ag_out[:].rearrange("(r p) (j q) -> p r j q", p=KC, q=R),
        )
        ks = [rank * (width // KC) + (g * hw) // KC + j
              for rank in range(NC) for j in range(nj)]
        return [(lt, ks)]

    # ---- layer 1 (K = 256 = 2 chunks, input replicated) ----
    xt = lpool.tile([KC, 2 * R], bf16, tag="lhs")
    nc.sync.dma_start(
        xt[:].rearrange("p (c r) -> p c r", r=R),
        aps["XT"].rearrange("(c p) r -> p c r", p=KC),
    )
    parts = [(xt, [0, 1])]

    for li in range(1, 6):
        width = SL
        nk = (2 * DIM if li == 1 else H) // KC
        hw = width // G
        wtiles = load_w(aps[f"W{li}"], nk, width)
        btile = bpool.tile([1, width], bf16, tag="bias")
        nc.sync.dma_start(btile[0:1, :], aps[f"b{li}"].unsqueeze(0))
        new_parts = []
        for g in range(G):
            pt = half_open(parts, wtiles, btile, width, hw, g)
            act = lrelu_half(pt, hw)
            new_parts.extend(gather_half(act, width, hw, g))
        parts = new_parts

    # ---- layer 6: full W6 on every core (no collective), K=4096, N=1024 ----
    # lrelu -> sigmoid -> mult/add straight into the policy-matrix tile M
    nk6 = H // KC
    b6tile = bpool.tile([1, OF], bf16, tag="bias")
    nc.sync.dma_start(b6tile[0:1, :], aps["b6"].unsqueeze(0))
    M = pipool.tile([R, OF], f32, tag="M")
    for nb in range(2):
        w6tiles = load_w(aps["W6"][:, nb * 512:(nb + 1) * 512], nk6, 512)
        pt = half_open(parts, w6tiles, b6tile, 512, 512, 0, bofs=nb * 512)
        z = sum_halves(pt, 512)
        sc = apool.tile([R, 512], f32, tag="lrelu_sc")
        nc.vector.tensor_scalar_mul(sc[:], z[:], SLOPE)
        lr = apool.tile([R, 512], f32, tag="lrelu_out")
        nc.vector.tensor_tensor(lr[:], z[:], sc[:], op=ALU.max)
        sg = apool.tile([R, 512], f32, tag="sig")
        nc.scalar.activation(sg[:], lr[:], AF.Sigmoid)
        nc.vector.tensor_scalar(
            M[:, nb * 512:(nb + 1) * 512], sg[:], mac[:, 0:1], mac[:, 1:2],
            op0=ALU.mult, op1=ALU.add,
        )

    # ---- power iteration: b <- M b, unnormalized ----
    # early iterations multiply in bf16 (self-correcting; only the final
    # step's precision survives), last iteration in fp32
    M3 = M[:].rearrange("p (r q) -> p r q", q=N)
    Mb = pipool.tile([R, OF], bf16, tag="Mb")
    nc.scalar.activation(Mb[:], M[:], AF.Copy)
    Mb3 = Mb[:].rearrange("p (r q) -> p r q", q=N)
    bv = pipool.tile([R, N], f32, tag="bv")
    nc.vector.reduce_sum(bv[:], M3, axis=AX.X)  # first step from b0 = ones
    for it in range(PI_ITERS):
        last = it == PI_ITERS - 1
        bb = bv[:].unsqueeze(1).broadcast_to((R, N, N))
        if last:
            tmp = pipool.tile([R, OF], f32, tag="pit")
            t3 = tmp[:].rearrange("p (r q) -> p r q", q=N)
            nc.vector.tensor_tensor(t3, M3, bb, op=ALU.mult)
        else:
            tmp = pipool.tile([R, OF], bf16, tag="pitb")
            t3 = tmp[:].rearrange("p (r q) -> p r q", q=N)
            nc.vector.tensor_tensor(t3, Mb3, bb, op=ALU.mult)
        bv = pipool.tile([R, N], f32, tag="bv")
        nc.vector.reduce_sum(bv[:], t3, axis=AX.X)

    # ---- deltas tail ----
    scr = tailp.tile([R, N], f32, tag="scr")
    d = tailp.tile([R, 1], f32, tag="d")
    nc.vector.tensor_tensor(scr[:], bv[:], dmask[:], op=ALU.mult)
    nc.vector.reduce_sum(d[:], scr[:], axis=AX.X)
    recipd = tailp.tile([R, 1], f32, tag="rd")
    nc.vector.reciprocal(recipd[:], d[:])
    recipE = tailp.tile([R, N], f32, tag="rE")
    nc.vector.reciprocal(recipE[:], bv[:])
    w01 = tailp.tile([R, 1], f32, tag="w01")
    nc.vector.reduce_sum(w01[:], t01[:], axis=AX.X)
    coef_s = tailp.tile([R, 1], f32, tag="cs")
    nc.vector.tensor_tensor(coef_s[:], w01[:], recipd[:], op=ALU.mult)
    scr2 = tailp.tile([R, N], f32, tag="scr2")
    c23 = tailp.tile([R, 1], f32, tag="c23")
    nc.vector.tensor_tensor(scr2[:], tt23[:], recipE[:], op=ALU.mult)
    nc.vector.reduce_sum(c23[:], scr2[:], axis=AX.X)
    coef = tailp.tile([R, B], f32, tag="coef")
    nc.vector.memset(coef[:], 0.0)
    nc.vector.tensor_copy(coef[0:32, 0:1], coef_s[0:32, :])
    nc.vector.tensor_copy(coef[32:64, 1:2], coef_s[32:64, :])
    nc.vector.tensor_copy(coef[0:32, 2:3], c23[0:32, :])
    nc.vector.tensor_copy(coef[32:64, 3:4], c23[32:64, :])
    pd = pst.tile([B, N], f32, tag="pd")
    nc.tensor.matmul(pd[:], coef[:], bv[:], start=True, stop=True)
    osb = tailp.tile([B, N], f32, tag="osb")
    nc.vector.tensor_copy(osb[:], pd[:])
    nc.sync.dma_start(aps["out"][:], osb[:])
    es.close()


def build():
    import concourse.bacc as bacc
    import concourse.mybir as mybir
    import concourse.tile as tile

    f32 = mybir.dt.float32
    bf16 = mybir.dt.bfloat16
    nc = bacc.Bacc("TRN2", target_bir_lowering=False, debug=False, num_devices=NC)
    shapes = {
        "XT": ([2 * DIM, R], bf16),
        "W1": ([2 * DIM, SL], bf16), "b1": ([SL], bf16),
        "W2": ([H, SL], bf16), "b2": ([SL], bf16),
        "W3": ([H, SL], bf16), "b3": ([SL], bf16),
        "W4": ([H, SL], bf16), "b4": ([SL], bf16),
        "W5": ([H, SL], bf16), "b5": ([SL], bf16),
        "W6": ([H, OF], bf16), "b6": ([OF], bf16),
        "T01": ([R, N], f32), "TT23": ([R, N], f32),
        "DMASK": ([R, N], f32), "MAC": ([R, 2], f32), "ID64": ([64, 64], bf16),
    }
    aps = {
        k: nc.dram_tensor(k, v[0], v[1], kind="ExternalInput").ap()
        for k, v in shapes.items()
    }
    aps["out"] = nc.dram_tensor("out", [B, N], f32, kind="ExternalOutput").ap()
    aps["warm"] = nc.dram_tensor("warm", [1, 8], bf16, kind="ExternalOutput").ap()
    with tile.TileContext(nc) as tc:
        _build_body(nc, tc, tile, mybir, aps)
    nc.compile()
    return nc


def prep_in_maps(inputs):
    import ml_dtypes
    f = np.float32
    bf = ml_dtypes.bfloat16
    E = np.asarray(inputs["batch_node_embeddings"], f)   # (B,N,D)
    T = np.asarray(inputs["batch_Ts"], f)                # (B,N,N)
    mult = np.asarray(inputs["mult_const_batch"], f).reshape(-1)[0]
    add = np.asarray(inputs["add_const_batch"], f).reshape(-1)[0]
    S = np.transpose(E, (1, 0, 2))                       # (N,B,D)
    G0 = np.concatenate([S[:, 0], S[:, 1]], axis=-1)     # (N, 2D)
    G1 = np.concatenate([S[:, 2], S[:, 3]], axis=-1)
    rows = np.concatenate([G0, G1], axis=0)              # (64, 256)
    common = {
        "XT": np.ascontiguousarray(rows.T).astype(bf),
        "T01": np.ascontiguousarray(np.concatenate([T[0], T[1]], axis=0)),
        "TT23": np.ascontiguousarray(np.concatenate([T[2].T, T[3].T], axis=0)),
        "DMASK": np.ascontiguousarray(np.tile(np.eye(N, dtype=f), (2, 1))),
        "MAC": np.ascontiguousarray(
            np.stack([np.full(R, mult, f), np.full(R, add, f)], axis=1)
        ),
        "ID64": np.eye(64, dtype=bf),
    }
    in_maps = []
    for c in range(NC):
        m = dict(common)
        for li in range(1, 6):
            W = np.asarray(inputs[f"W{li}"], f)
            b = np.asarray(inputs[f"b{li}"], f)
            m[f"W{li}"] = np.ascontiguousarray(W[:, c * SL:(c + 1) * SL]).astype(bf)
            m[f"b{li}"] = np.ascontiguousarray(b[c * SL:(c + 1) * SL]).astype(bf)
        m["W6"] = np.asarray(inputs["W6"], f).astype(bf)
        m["b6"] = np.asarray(inputs["b6"], f).astype(bf)
        in_maps.append(m)
    return in_maps


def kernel(**inputs):
    global _COMPILED, LAST_RESULTS
    from concourse import bass_utils

    if _COMPILED is None:
        _COMPILED = build()
    in_maps = prep_in_maps(inputs)
    res = bass_utils.run_bass_kernel_spmd(
        _COMPILED, in_maps, core_ids=list(range(NC))
    )
    LAST_RESULTS = res
    return np.asarray(res.results[0]["out"], np.float32)



# revision 9
# speedup vs baseline: 1.1603x; 1.1603x over previous
"""Trainium2 Bass kernel for nn_NisuyNN_90434831384984.

Math: the reference's stack+reshape makes MLP row (s,t,b) depend only on s
(b in {0,1}) or only on t (b in {2,3}) -- 64 unique rows through the MLP
produce 64 unique 32x32 policy matrices.  Note the reference applies
LeakyReLU to ALL six layers (including layer 6) before the sigmoid.

v2 layout:
  - L1..L4: Megatron column-split (512 cols/core), AllGather of the
    transposed bf16 activations after each layer.
  - L5: column-split, NO gather (output stays local).
  - L6: row-split (each core contracts its own 512 features against its
    W6 row-slice, with W6's columns host-permuted so the output rows are
    M^T); one fp32 ReduceScatter both sums the partials AND shards the 64
    rows 8-per-core for the tail.
  - Tail: per-core 8 rows; bias+LeakyReLU+sigmoid+scale on two [128,32]
    block tiles, assembled into two 128x128 block-diagonal matrices
    X=diag(M_r^T); power iteration replaced by 3 fp32 PE squarings
    (M^8 * ones == the baseline's 8 unnormalized steps; scale cancels in
    the delta ratios), then per-core delta coefficients and a tiny final
    AllGather with a summation on every core.
  - Weights are host-packed partition-major ([128, nk*width]) so every
    DMA moves multi-KB contiguous lines per partition.
  - First collective is AG1; gpsimd's queue holds only collectives so the
    CC doorbell (which gates the one-time ~42us collectives-init barrier)
    fires as early as possible.
"""

import numpy as np

DIM = 128
N = 32
B = 4
H = 4096
NC = 8          # cores
SL = H // NC    # 512 hidden slice per core
OF = N * N      # 1024 output features
R = 64          # unique MLP rows
KC = 128        # contraction chunk
SLOPE = 0.01
WSPIN = 0       # dummy warm-up matmuls after each layer (HAM experiment)

_COMPILED = None
LAST_RESULTS = None


def _build_body(nc, tc, tile, mybir, aps):
    f32 = mybir.dt.float32
    bf16 = mybir.dt.bfloat16
    AF = mybir.ActivationFunctionType
    ALU = mybir.AluOpType
    AX = mybir.AxisListType
    rg = [list(range(NC))]

    from contextlib import ExitStack
    es = ExitStack()
    cpool = es.enter_context(tc.tile_pool(name="consts", bufs=1))
    wpool = es.enter_context(tc.tile_pool(name="w", bufs=1))
    bpool = es.enter_context(tc.tile_pool(name="b", bufs=1))
    apool = es.enter_context(tc.tile_pool(name="act", bufs=2))
    atp = es.enter_context(tc.tile_pool(name="atT", bufs=2))
    lpool = es.enter_context(tc.tile_pool(name="lhs", bufs=2))
    tailp = es.enter_context(tc.tile_pool(name="tail", bufs=1))
    ps = es.enter_context(tc.tile_pool(name="ps", bufs=2, space="PSUM"))
    pst = es.enter_context(tc.tile_pool(name="pst", bufs=2, space="PSUM"))
    ps6 = es.enter_context(tc.tile_pool(name="ps6", bufs=1, space="PSUM"))
    tps = es.enter_context(tc.tile_pool(name="tps", bufs=2, space="PSUM"))
    dram = es.enter_context(tc.tile_pool(name="dram", bufs=1, space="DRAM"))

    # ---- weight/input DMAs on sync queue, in consumption order ----
    xt = wpool.tile([KC, 2 * R], bf16, tag="xt")
    nc.sync.dma_start(xt[:], aps["XT"][:])
    wts = {}
    bts = {}
    for li in range(1, 7):
        nk = 2 if li == 1 else (H // KC if li < 6 else 4)
        width = OF if li == 6 else SL
        wts[li] = wpool.tile([KC, nk * width], bf16, tag=f"w{li}",
                             name=f"wt{li}")
        nc.sync.dma_start(wts[li][:], aps[f"W{li}"][:])
        if li < 6:
            bts[li] = bpool.tile([1, SL], bf16, tag=f"b{li}", name=f"bt{li}")
            nc.sync.dma_start(bts[li][0:1, :], aps[f"b{li}"].unsqueeze(0))

    # ---- constants (scalar queue; keep gpsimd free for the CC doorbell) ----
    id64 = cpool.tile([64, 64], bf16)
    nc.scalar.dma_start(id64[:], aps["ID64"][:])
    id128f = cpool.tile([128, 128], f32)
    nc.scalar.dma_start(id128f[:], aps["ID128F"][:])
    bias6 = cpool.tile([128, N], f32)
    nc.scalar.dma_start(bias6[:], aps["BIAS6"][:])
    mac = cpool.tile([128, 2], f32)
    nc.scalar.dma_start(mac[:], aps["MAC"][:])
    t8 = cpool.tile([8, N], f32)
    nc.scalar.dma_start(t8[:], aps["T8"][:])
    tt8 = cpool.tile([8, N], f32)
    nc.scalar.dma_start(tt8[:], aps["TT8"][:])
    dm8 = cpool.tile([8, N], f32)
    nc.scalar.dma_start(dm8[:], aps["DM8"][:])
    sels = cpool.tile([8, B], f32)
    nc.scalar.dma_start(sels[:], aps["SELS"][:])
    selt = cpool.tile([8, B], f32)
    nc.scalar.dma_start(selt[:], aps["SELT"][:])
    onesb = cpool.tile([1, R], bf16)
    nc.vector.memset(onesb[:], 1.0)
    ones128 = cpool.tile([128, 1], f32)
    nc.vector.memset(ones128[:], 1.0)
    # block-diagonal tail matrices (memset early, filled after the RS)
    x1a = tailp.tile([128, 128], f32, tag="x1a")
    nc.vector.memset(x1a[:], 0.0)
    x1b = tailp.tile([128, 128], f32, tag="x1b")
    nc.vector.memset(x1b[:], 0.0)

    def layer_mm(chunks, wt, btile, li):
        """chunks: list of (lhs_ap, global_k).  Returns the [128, SL] psum
        with the two 64-wide column-group partial sums in rows [0:64] and
        [64:128]; bias is accumulated into group 0."""
        pt = ps.tile([2 * R, SL], f32, tag="ps", name=f"pt{li}")
        first = [True, True]
        n_h1 = sum(1 for i in range(len(chunks)) if i % 2 == 1)
        seen_h1 = 0
        for i, (lhs, k) in enumerate(chunks):
            h = i % 2
            if h == 1:
                seen_h1 += 1
            nc.tensor.matmul(
                pt[h * R:(h + 1) * R, :],
                lhs,
                wt[:, k * SL:(k + 1) * SL],
                start=first[h],
                stop=(h == 1 and seen_h1 == n_h1),
                tile_position=(0, h * R),
                skip_group_check=True,
            )
            first[h] = False
        nc.tensor.matmul(
            pt[0:R, :], onesb[0:1, :], btile[0:1, :],
            start=first[0], stop=True, tile_position=(0, 0),
            skip_group_check=True,
        )
        return pt

    def act_transpose(pt, li):
        """psum halves -> z -> LeakyReLU -> bf16 -> transposed att tile."""
        z0 = apool.tile([R, SL], f32, tag="z0", name=f"z0{li}")
        nc.scalar.activation(z0[:], pt[0:R, :], AF.Copy)
        z = apool.tile([R, SL], f32, tag="z", name=f"z{li}")
        nc.vector.tensor_tensor(z[:], z0[:], pt[R:2 * R, :], op=ALU.add)
        act = apool.tile([R, SL], bf16, tag="act", name=f"act{li}")
        nc.scalar.activation(act[:], z[:], AF.Lrelu, alpha=SLOPE)
        att = atp.tile([KC, 4 * R], bf16, tag="att", name=f"att{li}")
        for j in range(4):
            tp = pst.tile([KC, R], bf16, tag="pst", name=f"tp{li}_{j}")
            nc.tensor.transpose(tp[:], act[:, j * KC:(j + 1) * KC], id64[:])
            if j % 2 == 0:
                nc.vector.tensor_copy(att[:, j * R:(j + 1) * R], tp[:])
            else:
                nc.scalar.activation(att[:, j * R:(j + 1) * R], tp[:], AF.Copy)
        return att

    def gather(att, li):
        ag_in = dram.tile([KC, 4 * R], bf16, tag=f"agin{li}")
        nc.scalar.dma_start(ag_in[:], att[:])
        ag_out = dram.tile([NC * KC, 4 * R], bf16, tag=f"agout{li}",
                           addr_space="Shared")
        nc.gpsimd.collective_compute(
            "AllGather", ALU.bypass, replica_groups=rg,
            ins=[ag_in[:].opt()], outs=[ag_out[:].opt()],
        )
        engs = [nc.sync, nc.scalar, nc.gpsimd]
        chunks = []
        for r in range(NC):
            lt = lpool.tile([KC, 4 * R], bf16, tag=f"lt{r}", name=f"lt{li}_{r}")
            engs[r % 3].dma_start(lt[:], ag_out[r * KC:(r + 1) * KC, :])
            for j in range(4):
                chunks.append((lt[:, j * R:(j + 1) * R], r * 4 + j))
        return chunks

    # ---- L1 ----
    chunks = [(xt[:, 0:R], 0), (xt[:, R:2 * R], 1)]
    for li in range(1, 5):
        pt = layer_mm(chunks, wts[li], bts[li], li)
        att = act_transpose(pt, li)
        chunks = gather(att, li)

    # ---- L5 (no gather) ----
    pt5 = layer_mm(chunks, wts[5], bts[5], 5)
    att5 = act_transpose(pt5, 5)

    # ---- L6 row-split partial: z6 = a5_c^T-chunks @ W6p-rows ----
    pt6a = ps6.tile([R, SL], f32, tag="p6a")
    pt6b = ps6.tile([R, SL], f32, tag="p6b")
    for kc in range(4):
        lhs = att5[:, kc * R:(kc + 1) * R]
        nc.tensor.matmul(pt6a[:], lhs, wts[6][:, kc * OF:kc * OF + SL],
                         start=(kc == 0), stop=(kc == 3),
                         tile_position=(0, 0), skip_group_check=True)
        nc.tensor.matmul(pt6b[:], lhs, wts[6][:, kc * OF + SL:(kc + 1) * OF],
                         start=(kc == 0), stop=(kc == 3),
                         tile_position=(0, 0), skip_group_check=True)
    z6 = apool.tile([R, OF], f32, tag="z6", bufs=1)
    nc.vector.tensor_copy(z6[:, 0:SL], pt6a[:])
    nc.scalar.activation(z6[:, SL:OF], pt6b[:], AF.Copy)
    rs_in = dram.tile([R, OF], f32, tag="rsin")
    nc.sync.dma_start(rs_in[:], z6[:])
    rs_out = dram.tile([NC, OF], f32, tag="rsout")
    nc.gpsimd.collective_compute(
        "ReduceScatter", ALU.add, replica_groups=rg,
        ins=[rs_in[:].opt()], outs=[rs_out[:].opt()],
    )

    # ---- tail: 8 rows on this core ----
    # z-blocks [128, 32]: partition (rl*32 + j), free i holds z[rl, j*32+i]
    zza = tailp.tile([128, N], f32, tag="zza")
    zzb = tailp.tile([128, N], f32, tag="zzb")
    engs = [nc.sync, nc.scalar, nc.gpsimd]
    for rl in range(8):
        tgt = zza if rl < 4 else zzb
        po = (rl % 4) * 32
        engs[rl % 3].dma_start(
            tgt[po:po + 32, :],
            rs_out[rl:rl + 1, :].rearrange("r (j i) -> (r j) i", i=N),
        )

    def poltile(zz, name):
        zb = tailp.tile([128, N], f32, tag=f"zb_{name}")
        nc.vector.tensor_tensor(zb[:], zz[:], bias6[:], op=ALU.add)
        # LeakyReLU (mul+max keeps exact reference semantics)
        sc = tailp.tile([128, N], f32, tag=f"sc_{name}")
        nc.vector.tensor_scalar_mul(sc[:], zb[:], SLOPE)
        lr = tailp.tile([128, N], f32, tag=f"lr_{name}")
        nc.vector.tensor_tensor(lr[:], zb[:], sc[:], op=ALU.max)
        sg = tailp.tile([128, N], f32, tag=f"sg_{name}")
        nc.scalar.activation(sg[:], lr[:], AF.Sigmoid)
        pol = tailp.tile([128, N], f32, tag=f"pol_{name}")
        nc.vector.tensor_scalar(pol[:], sg[:], mac[:, 0:1], mac[:, 1:2],
                                op0=ALU.mult, op1=ALU.add)
        return pol

    pola = poltile(zza, "a")
    polb = poltile(zzb, "b")
    for rl in range(4):
        s = slice(rl * 32, (rl + 1) * 32)
        nc.vector.tensor_copy(x1a[s, s], pola[s, :])
        nc.scalar.activation(x1b[s, s], polb[s, :], AF.Copy)

    def streamT(x, name):
        y = tailp.tile([128, 128], f32, tag=f"y_{name}")
        nc.vector.transpose(y[:], x[:])
        return y

    y1a = streamT(x1a, "1a")
    y1b = streamT(x1b, "1b")

    def sq(x, y, name, want_y=True):
        px = tps.tile([128, 128], f32, tag="tps", name=f"px{name}")
        nc.tensor.matmul(px[:], y[:], x[:], start=True, stop=True)
        x2 = tailp.tile([128, 128], f32, tag=f"x_{name}")
        nc.vector.tensor_copy(x2[:], px[:])
        if not want_y:
            return x2, None
        py = tps.tile([128, 128], f32, tag="tps", name=f"py{name}")
        nc.tensor.matmul(py[:], x[:], y[:], start=True, stop=True)
        y2 = tailp.tile([128, 128], f32, tag=f"y_{name}")
        nc.scalar.activation(y2[:], py[:], AF.Copy)
        return x2, y2

    x2a, y2a = sq(x1a, y1a, "2a")
    x2b, y2b = sq(x1b, y1b, "2b")
    x4a, y4a = sq(x2a, y2a, "4a")
    x4b, y4b = sq(x2b, y2b, "4b")
    x8a, _ = sq(x4a, y4a, "8a", want_y=False)
    x8b, _ = sq(x4b, y4b, "8b", want_y=False)

    # bv = X8^T-columns summed = rowsums of M^8, per block position
    bva_ps = tps.tile([128, 1], f32, tag="tps", name="bvaps")
    nc.tensor.matmul(bva_ps[:], x8a[:], ones128[:], start=True, stop=True)
    bvb_ps = tps.tile([128, 1], f32, tag="tps", name="bvbps")
    nc.tensor.matmul(bvb_ps[:], x8b[:], ones128[:], start=True, stop=True)
    bva = tailp.tile([128, 1], f32, tag="bva")
    nc.vector.tensor_copy(bva[:], bva_ps[:])
    bvb = tailp.tile([128, 1], f32, tag="bvb")
    nc.scalar.activation(bvb[:], bvb_ps[:], AF.Copy)

    # partition->free layout flip via PE transpose + tiny DRAM round trip
    tba = tps.tile([1, 128], f32, tag="tps", name="tba")
    nc.tensor.transpose(tba[:], bva[:], id128f[:])
    tbb = tps.tile([1, 128], f32, tag="tps", name="tbb")
    nc.tensor.transpose(tbb[:], bvb[:], id128f[:])
    bvrow = tailp.tile([1, 2 * 128], f32, tag="bvrow")
    nc.vector.tensor_copy(bvrow[0:1, 0:128], tba[:])
    nc.scalar.activation(bvrow[0:1, 128:256], tbb[:], AF.Copy)
    bvd = dram.tile([1, 2 * 128], f32, tag="bvd")
    nc.sync.dma_start(bvd[:], bvrow[:])
    bv8 = tailp.tile([8, N], f32, tag="bv8")
    nc.scalar.dma_start(bv8[:], bvd[:].rearrange("o (r i) -> (o r) i", i=N))

    # delta coefficients
    recipE = tailp.tile([8, N], f32, tag="recipE")
    nc.vector.reciprocal(recipE[:], bv8[:])
    tmp = tailp.tile([8, N], f32, tag="tmp")
    nc.vector.tensor_tensor(tmp[:], bv8[:], dm8[:], op=ALU.mult)
    srcv = tailp.tile([8, 1], f32, tag="srcv")
    nc.vector.reduce_sum(srcv[:], tmp[:], axis=AX.X)
    rd = tailp.tile([8, 1], f32, tag="rd")
    nc.vector.reciprocal(rd[:], srcv[:])
    w01 = tailp.tile([8, 1], f32, tag="w01")
    nc.vector.reduce_sum(w01[:], t8[:], axis=AX.X)
    coefS = tailp.tile([8, 1], f32, tag="coefS")
    nc.vector.tensor_tensor(coefS[:], w01[:], rd[:], op=ALU.mult)
    tmp2 = tailp.tile([8, N], f32, tag="tmp2")
    nc.vector.tensor_tensor(tmp2[:], tt8[:], recipE[:], op=ALU.mult)
    c23 = tailp.tile([8, 1], f32, tag="c23")
    nc.vector.reduce_sum(c23[:], tmp2[:], axis=AX.X)
    t3 = tailp.tile([8, B], f32, tag="t3")
    nc.vector.tensor_scalar_mul(t3[:], sels[:], coefS[:, 0:1])
    t4 = tailp.tile([8, B], f32, tag="t4")
    nc.vector.tensor_scalar_mul(t4[:], selt[:], c23[:, 0:1])
    coefL = tailp.tile([8, B], f32, tag="coefL")
    nc.vector.tensor_tensor(coefL[:], t3[:], t4[:], op=ALU.add)
    pd_ps = tps.tile([B, N], f32, tag="tps", name="pdps")
    nc.tensor.matmul(pd_ps[:], coefL[:], bv8[:], start=True, stop=True)
    pd = tailp.tile([B, N], f32, tag="pd")
    nc.vector.tensor_copy(pd[:], pd_ps[:])

    # final gather of per-core partial deltas + sum on every core
    agf_in = dram.tile([B, N], f32, tag="agfin")
    nc.scalar.dma_start(agf_in[:], pd[:])
    agf_out = dram.tile([NC * B, N], f32, tag="agfout", addr_space="Shared")
    nc.gpsimd.collective_compute(
        "AllGather", ALU.bypass, replica_groups=rg,
        ins=[agf_in[:].opt()], outs=[agf_out[:].opt()],
    )
    pdall = tailp.tile([B, NC * N], f32, tag="pdall")
    nc.scalar.dma_start(
        pdall[:].rearrange("b (k j) -> b k j", j=N),
        agf_out[:].rearrange("(k b) j -> b k j", b=B),
    )
    osb = tailp.tile([B, N], f32, tag="osb")
    nc.vector.reduce_sum(
        osb[:],
        pdall[:].rearrange("b (k j) -> b j k", j=N),
        axis=AX.X,
    )
    nc.sync.dma_start(aps["out"][:], osb[:])
    es.close()


def build():
    import concourse.bacc as bacc
    import concourse.mybir as mybir
    import concourse.tile as tile

    f32 = mybir.dt.float32
    bf16 = mybir.dt.bfloat16
    nc = bacc.Bacc("TRN2", target_bir_lowering=False, debug=False, num_devices=NC)
    shapes = {
        "XT": ([KC, 2 * R], bf16),
        "W1": ([KC, 2 * SL], bf16), "b1": ([SL], bf16),
        "W2": ([KC, 32 * SL], bf16), "b2": ([SL], bf16),
        "W3": ([KC, 32 * SL], bf16), "b3": ([SL], bf16),
        "W4": ([KC, 32 * SL], bf16), "b4": ([SL], bf16),
        "W5": ([KC, 32 * SL], bf16), "b5": ([SL], bf16),
        "W6": ([KC, 4 * OF], bf16),
        "BIAS6": ([128, N], f32), "MAC": ([128, 2], f32),
        "T8": ([8, N], f32), "TT8": ([8, N], f32), "DM8": ([8, N], f32),
        "SELS": ([8, B], f32), "SELT": ([8, B], f32),
        "ID64": ([64, 64], bf16), "ID128F": ([128, 128], f32),
    }
    aps = {
        k: nc.dram_tensor(k, v[0], v[1], kind="ExternalInput").ap()
        for k, v in shapes.items()
    }
    aps["out"] = nc.dram_tensor("out", [B, N], f32, kind="ExternalOutput").ap()
    with tile.TileContext(nc) as tc:
        _build_body(nc, tc, tile, mybir, aps)
    nc.compile()
    return nc


def prep_in_maps(inputs):
    import ml_dtypes
    f = np.float32
    bf = ml_dtypes.bfloat16
    E = np.asarray(inputs["batch_node_embeddings"], f)   # (B,N,D)
    T = np.asarray(inputs["batch_Ts"], f)                # (B,N,N)
    mult = np.asarray(inputs["mult_const_batch"], f).reshape(-1)[0]
    add = np.asarray(inputs["add_const_batch"], f).reshape(-1)[0]
    S = np.transpose(E, (1, 0, 2))                       # (N,B,D)
    G0 = np.concatenate([S[:, 0], S[:, 1]], axis=-1)     # (32, 2D)
    G1 = np.concatenate([S[:, 2], S[:, 3]], axis=-1)
    rows = np.concatenate([G0, G1], axis=0)              # (64, 256)

    def packk(Wslice):
        # [nk*128, width] -> [128, nk*width], free index = k*width + n
        nk = Wslice.shape[0] // KC
        return np.ascontiguousarray(
            Wslice.reshape(nk, KC, -1).transpose(1, 0, 2).reshape(KC, -1)
        )

    perm = np.arange(OF).reshape(N, N).T.reshape(-1)     # perm[j*32+i] = i*32+j
    W6perm = np.asarray(inputs["W6"], f)[:, perm]
    b6p = np.asarray(inputs["b6"], f)[perm]

    common = {
        "XT": packk(rows.T).astype(bf),
        "BIAS6": np.ascontiguousarray(np.tile(b6p.reshape(N, N), (4, 1))),
        "MAC": np.ascontiguousarray(
            np.stack([np.full(128, mult, f), np.full(128, add, f)], axis=1)
        ),
        "ID64": np.eye(64, dtype=bf),
        "ID128F": np.eye(128, dtype=f),
    }
    in_maps = []
    for c in range(NC):
        m = dict(common)
        for li in range(1, 6):
            W = np.asarray(inputs[f"W{li}"], f)
            b = np.asarray(inputs[f"b{li}"], f)
            m[f"W{li}"] = packk(W[:, c * SL:(c + 1) * SL]).astype(bf)
            m[f"b{li}"] = np.ascontiguousarray(b[c * SL:(c + 1) * SL]).astype(bf)
        m["W6"] = packk(W6perm[c * SL:(c + 1) * SL, :]).astype(bf)
        bS = 0 if c < 4 else 1
        bT = 2 if c < 4 else 3
        t8 = np.zeros((8, N), f)
        tt8 = np.zeros((8, N), f)
        dm8 = np.zeros((8, N), f)
        sels = np.zeros((8, B), f)
        selt = np.zeros((8, B), f)
        for rl in range(8):
            s = (8 * c + rl) % N
            t8[rl] = T[bS][s, :]
            tt8[rl] = T[bT][:, s]
            dm8[rl, s] = 1.0
            sels[rl, bS] = 1.0
            selt[rl, bT] = 1.0
        m["T8"] = t8
        m["TT8"] = tt8
        m["DM8"] = dm8
        m["SELS"] = sels
        m["SELT"] = selt
        in_maps.append(m)
    return in_maps


def kernel(**inputs):
    global _COMPILED, LAST_RESULTS
    from concourse import bass_utils

    if _COMPILED is None:
        _COMPILED = build()
    in_maps = prep_in_maps(inputs)
    res = bass_utils.run_bass_kernel_spmd(
        _COMPILED, in_maps, core_ids=list(range(NC))
    )
    LAST_RESULTS = res
    return np.asarray(res.results[0]["out"], np.float32)
